# revision 1
# baseline (speedup 1.0000x reference)
# Trainium2 Bass kernel for LinearAttention (nn_LinearAttention_87686052315975).
#
# Reference computation (per batch element b of 16):
#   xf = x[b].reshape(512, 4096)                      # [c, l]
#   qkv = w_qkv @ xf                                  # [1536, l]
#   q, k, v split into 8 heads x 64 dims
#   k = softmax(k, axis=l)
#   context_h = k_h @ v_h^T                           # [64, 64]
#   out_h = context_h^T @ q_h                         # [64, l]
#   y = w_out @ concat(out_h) + b_out                 # [512, l]
#
# Sharding: data-parallel over batch. 16 batches / 8 cores = 2 per core.
# No collectives needed; each core produces its own output slice.
#
# Per-core kernel structure (per batch, l chunked by 512):
#   Pass A: q = w_q^T-form matmul (kept resident in SBUF, [512, 4096]);
#           kT/vT computed transposed (l on partitions) so the context
#           contraction over l maps onto the PE K dim;
#           E = exp(kT) (no max subtraction needed: |k| ~ N(0,1));
#           ctx_h[d, e] += E_h^T-contract-vT_h via matmul, with a ones
#           column appended to vT so column 64 accumulates rowsum(E).
#   Finalize: ctx_n = ctx * (1/s) per row; pack head pairs into a
#           block-diagonal [128, 128] lhsT via SBUF->SBUF DMA.
#   Pass B: out = ctxP^T-contract-q (one matmul per head pair);
#           y = w_out^T-form matmul + bias; DMA out.
#
# All big matmuls run as float32r (split-precision fp32, 1 cycle/row at
# N>=256 vs 4 for plain fp32). The small context matmuls (N=65) run at
# 4 cycles/row regardless; dtype for them is configurable.

import os
import numpy as np
from contextlib import ExitStack

import concourse.bass as bass
import concourse.bacc as bacc
import concourse.mybir as mybir
import concourse.tile as tile

# ---- problem constants (hardcoded per contract) ----
B, DIM, HGT, WID = 16, 512, 64, 64
L = HGT * WID            # 4096
HEADS, DH = 8, 64
HIDDEN = HEADS * DH      # 512
NCORES = 8
BPC = B // NCORES        # 2 batches per core
P = 128
CHUNK = 512
NCHUNK = L // CHUNK      # 8
KT = DIM // P            # 4 contraction tiles over channels
MT = DIM // P            # 4 output row tiles
LM = CHUNK // P          # 4 l-subtiles per chunk
NPAIR = HEADS // 2       # 4 head pairs
VW = DH + 2              # per-head vT width: 64 v cols + 2 ones cols (even N)

F32 = mybir.dt.float32
F32R = mybir.dt.float32r
MM_DT = mybir.dt.float32r     # dtype for the big (N=512) matmuls
CTX_DT = mybir.dt.float32r    # dtype for the small context matmuls


def _mm(ap, dt):
    return ap.bitcast(dt)


def build_kernel(ctx: ExitStack, tc: "tile.TileContext", x_in, wqkvT_in, woutT_in,
                 bias_in, y_out):
    nc = tc.nc

    wpool = ctx.enter_context(tc.tile_pool(name="weights", bufs=1))
    qpool = ctx.enter_context(tc.tile_pool(name="qres", bufs=1))
    xpool = ctx.enter_context(tc.tile_pool(name="xc", bufs=8))
    epool = ctx.enter_context(tc.tile_pool(name="ev", bufs=8))
    opool = ctx.enter_context(tc.tile_pool(name="osb", bufs=8))
    ypool = ctx.enter_context(tc.tile_pool(name="ysb", bufs=3))
    cpool = ctx.enter_context(tc.tile_pool(name="ctxacc", bufs=1))
    ppool = ctx.enter_context(tc.tile_pool(name="ctxp", bufs=2))
    psmm = ctx.enter_context(tc.tile_pool(name="psmm", bufs=4, space="PSUM"))
    psctx = ctx.enter_context(tc.tile_pool(name="psctx", bufs=2, space="PSUM"))

    # ---- load weights once ----
    wqkv_sb = []
    for k in range(KT):
        t = wpool.tile([P, 3 * HIDDEN], F32R, tag=f"wqkv{k}", name=f"wqkv{k}")
        nc.sync.dma_start(t[:], wqkvT_in[k * P:(k + 1) * P, :])
        wqkv_sb.append(t)
    wout_sb = []
    for k in range(KT):
        t = wpool.tile([P, DIM], F32R, tag=f"wout{k}", name=f"wout{k}")
        nc.sync.dma_start(t[:], woutT_in[k * P:(k + 1) * P, :])
        wout_sb.append(t)
    bias_sb = wpool.tile([P, MT], F32, tag="bias", name="bias")
    nc.sync.dma_start(bias_sb[:], bias_in[:])

    for b in range(BPC):
        # persistent q for this batch: 4 tiles [128, 4096]
        q_sb = [qpool.tile([P, L], F32R, tag=f"q{m}", name=f"q{m}") for m in range(MT)]
        # per-pair context accumulators [128, 132]
        ctx_acc = [cpool.tile([P, 2 * VW], F32, tag=f"ctxacc{p}", name=f"ctxacc{p}")
                   for p in range(NPAIR)]
        # block-diagonal lhsT tiles for pass B; zero-filled early so the
        # finalize chain stays short (HAM stays warm between passes)
        ctxP = []
        for p in range(NPAIR):
            t = ppool.tile([P, P], F32R, tag=f"p{p}", name=f"p{p}")
            nc.vector.tensor_scalar(t[:], wout_sb[0][:, 0:P], 0.0, None,
                                    mybir.AluOpType.mult)
            ctxP.append(t)

        # ---------------- Pass A ----------------
        for i in range(NCHUNK):
            ls = slice(i * CHUNK, (i + 1) * CHUNK)
            xc = []
            for k in range(KT):
                t = xpool.tile([P, CHUNK], F32R, tag="xc", name="xc")
                nc.sync.dma_start(t[:], x_in[b, k * P:(k + 1) * P, ls])
                xc.append(t)

            # q projection: q[o, l] for o-tile m
            for m in range(MT):
                ps = psmm.tile([P, CHUNK], F32, tag="mm", name="mm")
                for k in range(KT):
                    nc.tensor.matmul(
                        ps[:],
                        _mm(wqkv_sb[k][:, m * P:(m + 1) * P], MM_DT),
                        _mm(xc[k][:], MM_DT),
                        start=(k == 0), stop=(k == KT - 1))
                nc.vector.tensor_copy(q_sb[m][:, ls], ps[:])

            # kT/vT projection (l on partitions), exp, ones-append
            E_t, vT_t = [], []
            for lm in range(LM):
                # k half -> E = exp(kT)
                ps = psmm.tile([P, CHUNK], F32, tag="mm", name="mm")
                for k in range(KT):
                    nc.tensor.matmul(
                        ps[:],
                        _mm(xc[k][:, lm * P:(lm + 1) * P], MM_DT),
                        _mm(wqkv_sb[k][:, HIDDEN:2 * HIDDEN], MM_DT),
                        start=(k == 0), stop=(k == KT - 1))
                e = epool.tile([P, CHUNK], F32R, tag="E", name="E")
                nc.scalar.activation(e[:], ps[:],
                                     mybir.ActivationFunctionType.Exp)
                E_t.append(e)

                # v half -> vT with a ones column per head ([128, 8*65])
                ps = psmm.tile([P, CHUNK], F32, tag="mm", name="mm")
                for k in range(KT):
                    nc.tensor.matmul(
                        ps[:],
                        _mm(xc[k][:, lm * P:(lm + 1) * P], MM_DT),
                        _mm(wqkv_sb[k][:, 2 * HIDDEN:3 * HIDDEN], MM_DT),
                        start=(k == 0), stop=(k == KT - 1))
                v = epool.tile([P, HEADS * VW], F32R, tag="vT", name="vT")
                v_view = v[:].rearrange("p (h e) -> p h e", e=VW)
                nc.vector.tensor_copy(
                    v_view[:, :, 0:DH],
                    ps[:].rearrange("p (h e) -> p h e", e=DH))
                nc.vector.tensor_scalar(
                    v_view[:, :, DH:DH + 2],
                    ps[:].rearrange("p (h e) -> p h e", e=DH)[:, :, 0:2],
                    0.0, 1.0, mybir.AluOpType.mult, mybir.AluOpType.add)
                vT_t.append(v)

            # context accumulation, one matmul per head PAIR:
            # out[0:64, 0:66] = ctx_h0 (+rowsum col 64),
            # out[64:128, 66:132] = ctx_h1 (+rowsum col 130);
            # off-diagonal blocks are computed but never read.
            for p in range(NPAIR):
                pc = psctx.tile([P, 2 * VW], F32, tag="ctx", name="ctx")
                for lm in range(LM):
                    nc.tensor.matmul(
                        pc[:],
                        _mm(E_t[lm][:, p * P:(p + 1) * P], CTX_DT),
                        _mm(vT_t[lm][:, p * 2 * VW:(p + 1) * 2 * VW], CTX_DT),
                        start=(lm == 0), stop=(lm == LM - 1))
                if i == 0:
                    nc.vector.tensor_copy(ctx_acc[p][:], pc[:])
                else:
                    nc.vector.tensor_add(ctx_acc[p][:], ctx_acc[p][:], pc[:])

        # ---------------- Finalize: normalize into block-diag ctxP -------
        for p in range(NPAIR):
            acc = ctx_acc[p]
            nc.vector.reciprocal(acc[0:DH, DH:DH + 1], acc[0:DH, DH:DH + 1])
            nc.vector.reciprocal(acc[DH:P, 2 * VW - 2:2 * VW - 1],
                                 acc[DH:P, 2 * VW - 2:2 * VW - 1])
            nc.vector.tensor_scalar_mul(ctxP[p][0:DH, 0:DH],
                                        acc[0:DH, 0:DH],
                                        acc[0:DH, DH:DH + 1])
            nc.vector.tensor_scalar_mul(ctxP[p][DH:P, DH:P],
                                        acc[DH:P, VW:VW + DH],
                                        acc[DH:P, 2 * VW - 2:2 * VW - 1])

        # ---------------- Pass B ----------------
        for i in range(NCHUNK):
            ls = slice(i * CHUNK, (i + 1) * CHUNK)
            out_sb = []
            for p in range(NPAIR):
                ps = psmm.tile([P, CHUNK], F32, tag="mm", name="mm")
                nc.tensor.matmul(ps[:], _mm(ctxP[p][:], MM_DT),
                                 _mm(q_sb[p][:, ls], MM_DT),
                                 start=True, stop=True)
                o = opool.tile([P, CHUNK], F32R, tag="osb", name="osb")
                nc.scalar.copy(o[:], ps[:])
                out_sb.append(o)
            for m in range(MT):
                ps = psmm.tile([P, CHUNK], F32, tag="mm", name="mm")
                for k in range(KT):
                    nc.tensor.matmul(
                        ps[:],
                        _mm(wout_sb[k][:, m * P:(m + 1) * P], MM_DT),
                        _mm(out_sb[k][:], MM_DT),
                        start=(k == 0), stop=(k == KT - 1))
                y = ypool.tile([P, CHUNK], F32, tag="ysb", name="ysb")
                nc.vector.tensor_scalar_add(y[:], ps[:],
                                            bias_sb[:, m:m + 1])
                nc.sync.dma_start(y_out[b, m * P:(m + 1) * P, ls], y[:])


def build_module():
    nc = bacc.Bacc("TRN2", target_bir_lowering=False, debug=False,
                   num_devices=NCORES)
    x_in = nc.dram_tensor("x", [BPC, DIM, L], F32R, kind="ExternalInput")
    wqkvT_in = nc.dram_tensor("w_qkvT", [DIM, 3 * HIDDEN], F32R,
                              kind="ExternalInput")
    woutT_in = nc.dram_tensor("w_outT", [HIDDEN, DIM], F32R,
                              kind="ExternalInput")
    bias_in = nc.dram_tensor("bias", [P, MT], F32, kind="ExternalInput")
    y_out = nc.dram_tensor("y", [BPC, DIM, L], F32, kind="ExternalOutput")
    with tile.TileContext(nc) as tc:
        with ExitStack() as ctx:
            build_kernel(ctx, tc, x_in, wqkvT_in, woutT_in, bias_in, y_out)
    nc.compile()
    return nc


def make_in_maps(x, w_qkv, w_out, b_out):
    x = np.ascontiguousarray(x, dtype=np.float32).reshape(B, DIM, L)
    wqkvT = np.ascontiguousarray(np.asarray(w_qkv, dtype=np.float32).T)
    woutT = np.ascontiguousarray(np.asarray(w_out, dtype=np.float32).T)
    bias = np.ascontiguousarray(
        np.asarray(b_out, dtype=np.float32).reshape(MT, P).T)
    in_maps = []
    for c in range(NCORES):
        in_maps.append({
            "x": x[c * BPC:(c + 1) * BPC],
            "w_qkvT": wqkvT,
            "w_outT": woutT,
            "bias": bias,
        })
    return in_maps


_NC_CACHE = None


def kernel(x, w_qkv, w_out, b_out, *, trace=False, trace_kwargs=None):
    """Full inputs in, full output out. Shards batch across 8 NeuronCores."""
    global _NC_CACHE
    from concourse.bass_utils import run_bass_kernel_spmd

    if _NC_CACHE is None:
        _NC_CACHE = build_module()
    nc = _NC_CACHE

    in_maps = make_in_maps(x, w_qkv, w_out, b_out)
    kw = dict(trace_kwargs or {})
    res = run_bass_kernel_spmd(nc, in_maps, list(range(NCORES)),
                               trace=trace, **kw)
    y = np.empty((B, DIM, HGT, WID), dtype=np.float32)
    for c in range(NCORES):
        y[c * BPC:(c + 1) * BPC] = res.results[c]["y"].reshape(
            BPC, DIM, HGT, WID)
    kernel.last_results = res
    return y



# revision 5
# speedup vs baseline: 1.4379x; 1.4379x over previous
# Trainium2 Bass kernel for LinearAttention — v2 (q-path folded).
#
# Reference computation (per batch element b of 16):
#   qkv = w_qkv @ x[b]; q,k,v split into 8 heads x 64 dims
#   E = exp(k); ctx_h = (E_h/rowsum) @ v_h^T        # [64, 64]
#   y = w_out @ concat(ctx_h^T @ q_h) + b_out
#
# Key algebra: y = Wy @ x + b where
#   Wy = w_out @ blockdiag(ctx~_h^T) @ w_q   (per batch, [512, 512])
# so q never needs to be computed over l. Per batch:
#   Pass A: kT/vT projections (l on partitions), E = exp(kT), ctx
#           accumulation via head-pair matmuls with ones columns in vT
#           producing rowsums.
#   Fold:   ctx~ = ctx/rowsum; tmp_h = ctx~_h^T @ w_q_h;
#           WyT[c, y] = sum_h tmp_h^T-contract-w_outT  ([512, 512])
#   Pass Y: y = WyT^T-contract-x + bias; DMA out (fp16).
#
# All matmuls in fp16 (1 cycle/row at any N; fp32 PSUM accumulate).
# Data-parallel over batch: 16 batches / 8 cores = 2 per core.

import numpy as np
from contextlib import ExitStack

import concourse.bass as bass
import concourse.bacc as bacc
import concourse.mybir as mybir
import concourse.tile as tile

B, DIM, HGT, WID = 16, 512, 64, 64
L = HGT * WID            # 4096
HEADS, DH = 8, 64
HIDDEN = HEADS * DH      # 512
NCORES = 8
BPC = B // NCORES        # 2 batches per core
P = 128
CHUNK = 512
NCHUNK = L // CHUNK      # 8
KT = DIM // P            # 4 contraction tiles over channels
MT = DIM // P            # 4 output row tiles
LM = CHUNK // P          # 4 l-subtiles per chunk
NPAIR = HEADS // 2       # 4 head pairs
VW = DH + 2              # per-head vT width: 64 v cols + 2 ones cols

F32 = mybir.dt.float32
F16 = mybir.dt.float16


def build_kernel(ctx: ExitStack, tc: "tile.TileContext", x_in, wkvT_in, wq_in,
                 woT_in, bias_in, y_out):
    nc = tc.nc

    wpool = ctx.enter_context(tc.tile_pool(name="weights", bufs=1))
    xpool = ctx.enter_context(tc.tile_pool(name="xres", bufs=1))
    epool = ctx.enter_context(tc.tile_pool(name="ev", bufs=6))
    ypool = ctx.enter_context(tc.tile_pool(name="ysb", bufs=4))
    cpool = ctx.enter_context(tc.tile_pool(name="ctxacc", bufs=1))
    fpool = ctx.enter_context(tc.tile_pool(name="fold", bufs=1))
    wypool = ctx.enter_context(tc.tile_pool(name="wyt", bufs=1))
    psk = ctx.enter_context(tc.tile_pool(name="psk", bufs=2, space="PSUM"))
    psv = ctx.enter_context(tc.tile_pool(name="psv", bufs=2, space="PSUM"))
    psy = ctx.enter_context(tc.tile_pool(name="psy", bufs=2, space="PSUM"))
    psc = ctx.enter_context(tc.tile_pool(name="psc", bufs=2, space="PSUM"))

    # ---- weights, loaded once ----
    wkv_sb = []   # [128 c, 1024] = [wk^T | wv^T] cols per c-tile
    for k in range(KT):
        t = wpool.tile([P, 2 * HIDDEN], F16, tag=f"wkv{k}", name=f"wkv{k}")
        nc.sync.dma_start(t[:], wkvT_in[k * P:(k + 1) * P, :])
        wkv_sb.append(t)
    wq_sb = []    # [128 hid-d, 512 c] per pair tile (wq rows as-is)
    for p in range(NPAIR):
        t = wpool.tile([P, DIM], F16, tag=f"wq{p}", name=f"wq{p}")
        nc.sync.dma_start(t[:], wq_in[p * P:(p + 1) * P, :])
        wq_sb.append(t)
    woT_sb = []   # [128 hid-e, 512 y] per pair tile (w_out^T rows)
    for p in range(NPAIR):
        t = wpool.tile([P, DIM], F16, tag=f"wo{p}", name=f"wo{p}")
        nc.sync.dma_start(t[:], woT_in[p * P:(p + 1) * P, :])
        woT_sb.append(t)
    bias_sb = wpool.tile([P, MT], F32, tag="bias", name="bias")
    nc.sync.dma_start(bias_sb[:], bias_in[:])

    # ---- resident x tiles, fine-grained chunk DMAs ----
    x_sb = {}  # (b, kt, i) -> [128, 512] fp16
    for b in range(BPC):
        for k in range(KT):
            for i in range(NCHUNK):
                t = xpool.tile([P, CHUNK], F16, tag=f"x{b}_{k}_{i}",
                               name=f"x{b}_{k}_{i}")
                nc.sync.dma_start(
                    t[:], x_in[b, k * P:(k + 1) * P,
                               i * CHUNK:(i + 1) * CHUNK])
                x_sb[(b, k, i)] = t

    wyT_sb = {}  # (b, m) -> [128 c-sub, 512 y] fp16

    def pass_A(b):
        """kT/vT projections, exp, ctx accumulation for batch b."""
        ctx_acc = [cpool.tile([P, 2 * VW], F32, tag=f"ctxacc{p}",
                              name=f"ctxacc{p}") for p in range(NPAIR)]
        for i in range(NCHUNK):
            E_t, vT_t = [], []
            for lm in range(LM):
                lsl = slice(lm * P, (lm + 1) * P)
                kps = psk.tile([P, CHUNK], F32, tag="kps", name="kps")
                for k in range(KT):
                    nc.tensor.matmul(kps[:], x_sb[(b, k, i)][:, lsl],
                                     wkv_sb[k][:, 0:HIDDEN],
                                     start=(k == 0), stop=(k == KT - 1))
                vps = psv.tile([P, CHUNK], F32, tag="vps", name="vps")
                for k in range(KT):
                    nc.tensor.matmul(vps[:], x_sb[(b, k, i)][:, lsl],
                                     wkv_sb[k][:, HIDDEN:2 * HIDDEN],
                                     start=(k == 0), stop=(k == KT - 1))
                e = epool.tile([P, CHUNK], F16, tag="E", name="E")
                nc.scalar.activation(e[:], kps[:],
                                     mybir.ActivationFunctionType.Exp)
                E_t.append(e)
                v = epool.tile([P, HEADS * VW], F16, tag="vT", name="vT")
                v_view = v[:].rearrange("p (h e) -> p h e", e=VW)
                nc.vector.tensor_copy(
                    v_view[:, :, 0:DH],
                    vps[:].rearrange("p (h e) -> p h e", e=DH))
                nc.vector.tensor_scalar(
                    v_view[:, :, DH:DH + 2],
                    vps[:].rearrange("p (h e) -> p h e", e=DH)[:, :, 0:2],
                    0.0, 1.0, mybir.AluOpType.mult, mybir.AluOpType.add)
                vT_t.append(v)

            # ctx per head pair: out[0:64, 0:66] = ctx_h0 (+rowsum col 64),
            # out[64:128, 66:132] = ctx_h1 (+rowsum col 130)
            for p in range(NPAIR):
                pc = psc.tile([P, 2 * VW], F32, tag="ctx", name="ctx")
                for lm in range(LM):
                    nc.tensor.matmul(
                        pc[:], E_t[lm][:, p * P:(p + 1) * P],
                        vT_t[lm][:, p * 2 * VW:(p + 1) * 2 * VW],
                        start=(lm == 0), stop=(lm == LM - 1))
                if i == 0:
                    nc.vector.tensor_copy(ctx_acc[p][:], pc[:])
                else:
                    nc.vector.tensor_add(ctx_acc[p][:], ctx_acc[p][:], pc[:])
        return ctx_acc

    def fold(b, ctx_acc):
        """WyT = (w_out @ blockdiag(ctx~^T) @ w_q)^T for batch b."""
        tmp_sb = []
        for p in range(NPAIR):
            acc = ctx_acc[p]
            # 1/rowsum per head-half
            nc.vector.reciprocal(acc[0:DH, DH:DH + 1], acc[0:DH, DH:DH + 1])
            nc.vector.reciprocal(acc[DH:P, 2 * VW - 2:2 * VW - 1],
                                 acc[DH:P, 2 * VW - 2:2 * VW - 1])
            ctxn = fpool.tile([P, DH], F16, tag=f"ctxn{p}", name=f"ctxn{p}")
            nc.vector.tensor_scalar_mul(ctxn[0:DH, :], acc[0:DH, 0:DH],
                                        acc[0:DH, DH:DH + 1])
            nc.vector.tensor_scalar_mul(ctxn[DH:P, :], acc[DH:P, VW:VW + DH],
                                        acc[DH:P, 2 * VW - 2:2 * VW - 1])
            # tmp_h = ctx~_h^T @ wq_h, heads 2p / 2p+1 stacked on partitions
            tps = psk.tile([P, CHUNK], F32, tag="kps", name="kps")
            nc.tensor.matmul(tps[0:DH, :], ctxn[0:DH, :], wq_sb[p][0:DH, :],
                             start=True, stop=True, tile_position=(0, 0))
            nc.tensor.matmul(tps[DH:P, :], ctxn[DH:P, :], wq_sb[p][DH:P, :],
                             start=True, stop=True, tile_position=(DH, DH))
            t = fpool.tile([P, CHUNK], F16, tag=f"tmp{p}", name=f"tmp{p}")
            nc.scalar.copy(t[:], tps[:])
            tmp_sb.append(t)
        for m in range(MT):
            wps = psv.tile([P, CHUNK], F32, tag="vps", name="vps")
            for p in range(NPAIR):
                nc.tensor.matmul(wps[:], tmp_sb[p][:, m * P:(m + 1) * P],
                                 woT_sb[p][:],
                                 start=(p == 0), stop=(p == NPAIR - 1))
            t = wypool.tile([P, DIM], F16, tag=f"wyt{b}_{m}",
                            name=f"wyt{b}_{m}")
            nc.vector.tensor_copy(t[:], wps[:])
            wyT_sb[(b, m)] = t

    def pass_Y(b):
        """y = WyT^T @ x + bias for batch b; DMA out as fp16."""
        for i in range(NCHUNK):
            ls = slice(i * CHUNK, (i + 1) * CHUNK)
            for u in range(MT):
                yps = psy.tile([P, CHUNK], F32, tag="yps", name="yps")
                for k in range(KT):
                    nc.tensor.matmul(yps[:],
                                     wyT_sb[(b, k)][:, u * P:(u + 1) * P],
                                     x_sb[(b, k, i)][:],
                                     start=(k == 0), stop=(k == KT - 1))
                y = ypool.tile([P, CHUNK], F16, tag="ysb", name="ysb")
                nc.vector.tensor_scalar_add(y[:], yps[:],
                                            bias_sb[:, u:u + 1])
                nc.sync.dma_start(y_out[b, u * P:(u + 1) * P, ls], y[:])

    for b in range(BPC):
        ctx_acc = pass_A(b)
        fold(b, ctx_acc)
        pass_Y(b)


def build_module():
    nc = bacc.Bacc("TRN2", target_bir_lowering=False, debug=False,
                   num_devices=NCORES)
    x_in = nc.dram_tensor("x", [BPC, DIM, L], F16, kind="ExternalInput")
    wkvT_in = nc.dram_tensor("w_kvT", [DIM, 2 * HIDDEN], F16,
                             kind="ExternalInput")
    wq_in = nc.dram_tensor("w_q", [HIDDEN, DIM], F16, kind="ExternalInput")
    woT_in = nc.dram_tensor("w_oT", [HIDDEN, DIM], F16, kind="ExternalInput")
    bias_in = nc.dram_tensor("bias", [P, MT], F32, kind="ExternalInput")
    y_out = nc.dram_tensor("y", [BPC, DIM, L], F16, kind="ExternalOutput")
    with tile.TileContext(nc) as tc:
        with ExitStack() as ctx:
            build_kernel(ctx, tc, x_in, wkvT_in, wq_in, woT_in, bias_in,
                         y_out)
    nc.compile()
    return nc


def make_in_maps(x, w_qkv, w_out, b_out):
    x = np.ascontiguousarray(x, dtype=np.float32).reshape(B, DIM, L)
    x16 = x.astype(np.float16)
    w_qkv = np.asarray(w_qkv, dtype=np.float32)
    wq = np.ascontiguousarray(w_qkv[0:HIDDEN]).astype(np.float16)
    wkvT = np.ascontiguousarray(
        np.concatenate([w_qkv[HIDDEN:2 * HIDDEN].T,
                        w_qkv[2 * HIDDEN:3 * HIDDEN].T], axis=1)
    ).astype(np.float16)
    woT = np.ascontiguousarray(
        np.asarray(w_out, dtype=np.float32).T).astype(np.float16)
    bias = np.ascontiguousarray(
        np.asarray(b_out, dtype=np.float32).reshape(MT, P).T)
    in_maps = []
    for c in range(NCORES):
        in_maps.append({
            "x": x16[c * BPC:(c + 1) * BPC],
            "w_kvT": wkvT,
            "w_q": wq,
            "w_oT": woT,
            "bias": bias,
        })
    return in_maps


_NC_CACHE = None


def kernel(x, w_qkv, w_out, b_out, *, trace=False, trace_kwargs=None):
    """Full inputs in, full output out. Shards batch across 8 NeuronCores."""
    global _NC_CACHE
    from concourse.bass_utils import run_bass_kernel_spmd

    if _NC_CACHE is None:
        _NC_CACHE = build_module()
    nc = _NC_CACHE

    in_maps = make_in_maps(x, w_qkv, w_out, b_out)
    kw = dict(trace_kwargs or {})
    res = run_bass_kernel_spmd(nc, in_maps, list(range(NCORES)),
                               trace=trace, **kw)
    y = np.empty((B, DIM, HGT, WID), dtype=np.float32)
    for c in range(NCORES):
        y[c * BPC:(c + 1) * BPC] = res.results[c]["y"].astype(
            np.float32).reshape(BPC, DIM, HGT, WID)
    kernel.last_results = res
    return y
